# revision 11
# baseline (speedup 1.0000x reference)
"""AttnDecoderRNN single-step decoder on 8 Trainium2 NeuronCores.

Model (batch=1): embedding row -> Bahdanau attention over 25 encoder states
-> combine linear + relu -> GRU cell -> vocab projection (V=50257)
-> log_softmax.  Returns (log_probs[1,V], h_new[1,1,H], attn_weights[1,ML]).

Everything is a matrix-vector product, so the kernel streams each weight
matrix through SBUF once and contracts it with a partition-replicated input
vector using the DVE fused tensor_tensor_reduce (mult + free-dim add-reduce
with per-partition bias seed).  Sharding (tensor parallel over output rows):

  - W_comb rows, GRU gate rows, and W_out/vocab rows are split 8 ways.
  - Two 1KB AllGathers stitch the GRU input x and h_new back together;
    one 256B AllGather shares per-core (max, sumexp) so every core can
    finish log_softmax locally (streamed logsumexp merge).
  - Vocab padded 50257 -> 51200 = 8*128*50; pad rows get bias -1e30 so
    they vanish from the softmax; host slices them off.
"""

import functools

import numpy as np

H = 2048
V = 50257
ML = 25
NC = 8
TPC = 50                 # vocab tiles per core (free-dim columns of logits)
VP = 128 * TPC           # 6400 vocab rows per core
VPAD = NC * VP           # 51200
SL = H // NC             # 256: per-core slice of H-sized outputs (comb, gates)
NEG = -1.0e30


def _build_nc():
    import concourse.bacc as bacc
    import concourse.bass_isa as bass_isa
    import concourse.mybir as mybir
    import concourse.tile as tile

    f32 = mybir.dt.float32
    Alu = mybir.AluOpType
    Act = mybir.ActivationFunctionType
    RG = [list(range(NC))]

    nc = bacc.Bacc(
        "TRN2",
        target_bir_lowering=False,
        debug=False,
        enable_asserts=True,
        num_devices=NC,
    )

    def din(name, shape):
        return nc.dram_tensor(name, shape, f32, kind="ExternalInput").ap()

    def dout(name, shape):
        return nc.dram_tensor(name, shape, f32, kind="ExternalOutput").ap()

    emb_row = din("emb_row", [1, H])
    h0 = din("h0", [1, H])
    h0c = din("h0c", [SL])
    enc = din("enc", [ML, H])
    W_attn = din("W_attn", [ML, 2 * H])
    b_attn = din("b_attn", [ML])
    W_comb = din("W_comb", [SL, 2 * H])
    b_comb = din("b_comb", [SL])
    W_ih = din("W_ih", [3 * SL, H])
    b_ih = din("b_ih", [3 * SL])
    W_hh = din("W_hh", [3 * SL, H])
    b_hh = din("b_hh", [3 * SL])
    W_out = din("W_out", [VP, H])
    b_out = din("b_out", [VP])

    out_logp = dout("out_logp", [VP])
    out_hnew = dout("out_hnew", [SL])
    out_attnw = dout("out_attnw", [ML])

    def amr(wtile, xrep, acc):
        # acc = sum(wtile * xrep) per partition (seed=0); wtile clobbered.
        nc.vector.affine_mul_reduce(
            out=wtile,
            accum_out=acc,
            in0=wtile,
            in1=xrep,
            scale=1.0,
            bias=0.0,
        )

    with tile.TileContext(nc) as tc:
        with tc.tile_pool(name="consts", bufs=1) as consts, \
             tc.tile_pool(name="wstream", bufs=5) as wpool, \
             tc.tile_pool(name="work", bufs=1) as work, \
             tc.tile_pool(name="pp", bufs=1, space="PSUM") as pp, \
             tc.tile_pool(name="dram", bufs=1, space="DRAM") as dram:

            # ---------------- attention (identical on every core) ----------
            attn_in = work.tile([128, 2 * H], f32)
            nc.scalar.dma_start(attn_in[:, 0:H], emb_row.broadcast_to((128, H)))
            nc.scalar.dma_start(attn_in[:, H:2 * H], h0.broadcast_to((128, H)))

            wat = consts.tile([ML, 2 * H], f32)
            nc.sync.dma_start(wat[:], W_attn[:])
            enc_sb = consts.tile([ML, H], f32)
            nc.sync.dma_start(enc_sb[:], enc[:])
            bat = consts.tile([ML, 1], f32)
            nc.scalar.dma_start(bat[:], b_attn.rearrange("(p u) -> p u", u=1))

            # softmax over 25 values on the partition axis, padded to 32
            # (partition_all_reduce needs channels % 32 == 0; pads hold
            # -1e30 so they drop out of max and contribute exp() = 0).
            scores = work.tile([32, 1], f32)
            nc.vector.memset(scores[:], NEG)
            amr(wat[:], attn_in[0:ML, :], scores[0:ML, :])
            nc.vector.tensor_add(scores[0:ML, :], scores[0:ML, :], bat[:])

            mx = work.tile([32, 1], f32)
            nc.gpsimd.partition_all_reduce(mx[:], scores[:], channels=32,
                                           reduce_op=bass_isa.ReduceOp.max)
            negm = work.tile([32, 1], f32)
            nc.vector.tensor_scalar_mul(negm[:], mx[:], -1.0)
            ex = work.tile([32, 1], f32)
            nc.scalar.activation(ex[:], scores[:], Act.Exp, bias=negm[:])
            sm = work.tile([32, 1], f32)
            nc.gpsimd.partition_all_reduce(sm[:], ex[:], channels=32,
                                           reduce_op=bass_isa.ReduceOp.add)
            rs = work.tile([32, 1], f32)
            nc.vector.reciprocal(rs[:], sm[:])
            aw = work.tile([32, 1], f32)
            nc.vector.tensor_mul(aw[:], ex[:], rs[:])
            nc.scalar.dma_start(out_attnw.rearrange("(p u) -> p u", u=1),
                                aw[0:ML, :])

            # attn_applied = aw @ enc  on the PE (K=25, M=1, N=512 x4)
            ap_ps = pp.tile([1, H], f32)
            for j in range(4):
                nc.tensor.matmul(ap_ps[0:1, j * 512:(j + 1) * 512],
                                 lhsT=aw[0:ML, 0:1],
                                 rhs=enc_sb[:, j * 512:(j + 1) * 512],
                                 start=True, stop=True)
            cvec = work.tile([1, H], f32)
            nc.scalar.copy(cvec[:], ap_ps[:])

            comb_in = work.tile([128, 2 * H], f32)
            nc.scalar.dma_start(comb_in[:, 0:H], emb_row.broadcast_to((128, H)))
            nc.gpsimd.partition_broadcast(comb_in[:, H:2 * H], cvec[0:1, :],
                                          channels=128)

            # h0 replicated (needed later for gate-h matvecs; off critical path)
            h0_rep = work.tile([128, H], f32)
            nc.scalar.dma_start(h0_rep[:], h0.broadcast_to((128, H)))

            # ---------------- combine linear (sharded rows) -----------------
            bcomb = consts.tile([128, 2], f32)
            nc.scalar.dma_start(bcomb[:], b_comb.rearrange("(p u) -> p u", u=2))
            yx = work.tile([128, 2], f32)
            wcv = W_comb.rearrange("(p u) d -> u p d", p=128, u=2)
            for u in range(2):
                wc = wpool.tile([128, 2 * H], f32, tag="w", name=f"wc{u}")
                nc.sync.dma_start(wc[:], wcv[u])
                amr(wc[:], comb_in[:], yx[:, u:u + 1])
            nc.vector.tensor_add(yx[:], yx[:], bcomb[:])
            xs = work.tile([128, 2], f32)
            nc.vector.tensor_scalar_max(xs[:], yx[:], 0.0)

            ccx_in = dram.tile([SL], f32)
            nc.scalar.dma_start(ccx_in.rearrange("(p u) -> p u", u=2), xs[:])
            ccx_out = dram.tile([H], f32, addr_space="Shared")
            nc.gpsimd.collective_compute("AllGather", Alu.bypass,
                                         replica_groups=RG,
                                         ins=[ccx_in[:]], outs=[ccx_out[:]])
            x_rep = work.tile([128, H], f32)
            nc.scalar.dma_start(
                x_rep[:],
                ccx_out.rearrange("(a h) -> a h", a=1).broadcast_to((128, H)))

            # ---------------- GRU cell (sharded gate rows) ------------------
            bih = consts.tile([128, 3, 2], f32)
            nc.scalar.dma_start(bih[:], b_ih.rearrange("(g p u) -> p g u",
                                                       g=3, p=128, u=2))
            bhh = consts.tile([128, 3, 2], f32)
            nc.scalar.dma_start(bhh[:], b_hh.rearrange("(g p u) -> p g u",
                                                       g=3, p=128, u=2))
            gi = work.tile([128, 6], f32)
            gh = work.tile([128, 6], f32)
            wihv = W_ih.rearrange("(g p u) h -> g p u h", g=3, p=128, u=2)
            whhv = W_hh.rearrange("(g p u) h -> g p u h", g=3, p=128, u=2)
            for g in range(3):
                wi = wpool.tile([128, 2, H], f32, tag="w", name=f"wih{g}")
                nc.sync.dma_start(wi[:], wihv[g])
                for u in range(2):
                    c = 2 * g + u
                    amr(wi[:, u, :], x_rep[:], gi[:, c:c + 1])
            nc.vector.tensor_add(gi[:], gi[:],
                                 bih.rearrange("p a b -> p (a b)"))
            for g in range(3):
                wh = wpool.tile([128, 2, H], f32, tag="w", name=f"whh{g}")
                nc.sync.dma_start(wh[:], whhv[g])
                for u in range(2):
                    c = 2 * g + u
                    amr(wh[:, u, :], h0_rep[:], gh[:, c:c + 1])
            nc.vector.tensor_add(gh[:], gh[:],
                                 bhh.rearrange("p a b -> p (a b)"))

            rt = work.tile([128, 2], f32)
            nc.vector.tensor_add(rt[:], gi[:, 0:2], gh[:, 0:2])
            r = work.tile([128, 2], f32)
            nc.scalar.activation(r[:], rt[:], Act.Sigmoid)
            zt = work.tile([128, 2], f32)
            nc.vector.tensor_add(zt[:], gi[:, 2:4], gh[:, 2:4])
            z = work.tile([128, 2], f32)
            nc.scalar.activation(z[:], zt[:], Act.Sigmoid)
            nt = work.tile([128, 2], f32)
            nc.vector.tensor_mul(nt[:], r[:], gh[:, 4:6])
            nc.vector.tensor_add(nt[:], nt[:], gi[:, 4:6])
            n = work.tile([128, 2], f32)
            nc.scalar.activation(n[:], nt[:], Act.Tanh)

            h0cs = work.tile([128, 2], f32)
            nc.scalar.dma_start(h0cs[:], h0c.rearrange("(p u) -> p u", u=2))
            hn = work.tile([128, 2], f32)
            nc.vector.tensor_sub(hn[:], h0cs[:], n[:])
            nc.vector.tensor_mul(hn[:], hn[:], z[:])
            nc.vector.tensor_add(hn[:], hn[:], n[:])
            nc.scalar.dma_start(out_hnew.rearrange("(p u) -> p u", u=2), hn[:])

            cch_in = dram.tile([SL], f32)
            nc.scalar.dma_start(cch_in.rearrange("(p u) -> p u", u=2), hn[:])
            cch_out = dram.tile([H], f32, addr_space="Shared")
            nc.gpsimd.collective_compute("AllGather", Alu.bypass,
                                         replica_groups=RG,
                                         ins=[cch_in[:]], outs=[cch_out[:]])
            hn_rep = work.tile([128, H], f32)
            nc.scalar.dma_start(
                hn_rep[:],
                cch_out.rearrange("(a h) -> a h", a=1).broadcast_to((128, H)))

            # ---------------- vocab projection (sharded) --------------------
            bout = consts.tile([128, TPC], f32)
            nc.scalar.dma_start(bout[:], b_out.rearrange("(p t) -> p t", t=TPC))
            lg = work.tile([128, TPC], f32)
            wov = W_out.rearrange("(p t) h -> p t h", p=128, t=TPC)
            TB = 2
            for t0 in range(0, TPC, TB):
                wo = wpool.tile([128, TB, H], f32, tag="w", name=f"wo{t0}")
                nc.sync.dma_start(wo[:], wov[:, t0:t0 + TB, :])
                for j in range(TB):
                    t = t0 + j
                    amr(wo[:, j, :], hn_rep[:], lg[:, t:t + 1])
            nc.vector.tensor_add(lg[:], lg[:], bout[:])

            # ---------------- streamed log_softmax --------------------------
            mloc = work.tile([128, 1], f32)
            nc.vector.reduce_max(mloc[:], lg[:], axis=mybir.AxisListType.X)
            mcore = work.tile([128, 1], f32)
            nc.gpsimd.partition_all_reduce(mcore[:], mloc[:], channels=128,
                                           reduce_op=bass_isa.ReduceOp.max)
            negmc = work.tile([128, 1], f32)
            nc.vector.tensor_scalar_mul(negmc[:], mcore[:], -1.0)
            es = work.tile([128, TPC], f32)
            srow = work.tile([128, 1], f32)
            nc.scalar.activation(es[:], lg[:], Act.Exp, bias=negmc[:],
                                 accum_out=srow[:])
            score_s = work.tile([128, 1], f32)
            nc.gpsimd.partition_all_reduce(score_s[:], srow[:], channels=128,
                                           reduce_op=bass_isa.ReduceOp.add)

            pk = work.tile([1, 8], f32)
            nc.vector.memset(pk[:], 0.0)
            nc.vector.tensor_copy(pk[:, 0:1], mcore[0:1, 0:1])
            nc.vector.tensor_copy(pk[:, 1:2], score_s[0:1, 0:1])
            ccms_in = dram.tile([8], f32)
            nc.scalar.dma_start(ccms_in.rearrange("(a k) -> a k", a=1), pk[:])
            ccms_out = dram.tile([8 * NC], f32, addr_space="Shared")
            nc.gpsimd.collective_compute("AllGather", Alu.bypass,
                                         replica_groups=RG,
                                         ins=[ccms_in[:]], outs=[ccms_out[:]])
            msv = ccms_out.rearrange("(r k) -> k r", k=8)
            msm = work.tile([1, NC], f32)
            nc.scalar.dma_start(msm[:], msv[0:1, :])
            mss = work.tile([1, NC], f32)
            nc.scalar.dma_start(mss[:], msv[1:2, :])

            gm = work.tile([1, 1], f32)
            nc.vector.reduce_max(gm[:], msm[:], axis=mybir.AxisListType.X)
            ngm = work.tile([1, 1], f32)
            nc.vector.tensor_scalar_mul(ngm[:], gm[:], -1.0)
            emv = work.tile([1, NC], f32)
            nc.scalar.activation(emv[:], msm[:], Act.Exp, bias=ngm[:])
            dsv = work.tile([1, NC], f32)
            nc.vector.tensor_mul(dsv[:], emv[:], mss[:])
            ds = work.tile([1, 1], f32)
            nc.vector.reduce_sum(ds[:], dsv[:], axis=mybir.AxisListType.X)
            ld = work.tile([1, 1], f32)
            nc.scalar.activation(ld[:], ds[:], Act.Ln)
            nlz = work.tile([1, 1], f32)
            nc.vector.tensor_add(nlz[:], ld[:], gm[:])
            nc.vector.tensor_scalar_mul(nlz[:], nlz[:], -1.0)
            nlz_rep = work.tile([128, 1], f32)
            nc.gpsimd.partition_broadcast(nlz_rep[:], nlz[0:1, :], channels=128)

            logp = work.tile([128, TPC], f32)
            nc.vector.tensor_scalar_add(logp[:], lg[:], nlz_rep[:])
            nc.scalar.dma_start(out_logp.rearrange("(p t) -> p t", t=TPC),
                                logp[:])

    nc.compile()
    return nc


@functools.lru_cache(maxsize=1)
def _get_nc():
    return _build_nc()


def _per_core_inputs(inputs):
    f = np.float32
    emb = np.asarray(inputs["emb"], f)
    tok = int(np.asarray(inputs["input_tok"]).reshape(-1)[0])
    emb_row = np.ascontiguousarray(emb[tok:tok + 1, :])
    h0 = np.asarray(inputs["hidden"], f).reshape(1, H)
    enc = np.asarray(inputs["encoder_outputs"], f)
    W_attn = np.asarray(inputs["W_attn"], f)
    b_attn = np.asarray(inputs["b_attn"], f)
    W_comb = np.asarray(inputs["W_comb"], f)
    b_comb = np.asarray(inputs["b_comb"], f)
    W_ih = np.asarray(inputs["W_ih"], f)
    b_ih = np.asarray(inputs["b_ih"], f)
    W_hh = np.asarray(inputs["W_hh"], f)
    b_hh = np.asarray(inputs["b_hh"], f)
    W_out = np.asarray(inputs["W_out"], f)
    b_out = np.asarray(inputs["b_out"], f)

    in_maps = []
    for c in range(NC):
        s = slice(c * SL, (c + 1) * SL)
        gsl = [slice(g * H + c * SL, g * H + (c + 1) * SL) for g in range(3)]
        v0, v1 = c * VP, (c + 1) * VP
        if v1 <= V:
            woc = W_out[v0:v1]
            boc = b_out[v0:v1]
        else:
            nreal = max(V - v0, 0)
            woc = np.zeros((VP, H), f)
            woc[:nreal] = W_out[v0:V]
            boc = np.full((VP,), NEG, f)
            boc[:nreal] = b_out[v0:V]
        in_maps.append({
            "emb_row": emb_row,
            "h0": h0,
            "h0c": np.ascontiguousarray(h0[0, s]),
            "enc": enc,
            "W_attn": W_attn,
            "b_attn": b_attn,
            "W_comb": np.ascontiguousarray(W_comb[s]),
            "b_comb": np.ascontiguousarray(b_comb[s]),
            "W_ih": np.ascontiguousarray(np.concatenate([W_ih[g] for g in gsl])),
            "b_ih": np.ascontiguousarray(np.concatenate([b_ih[g] for g in gsl])),
            "W_hh": np.ascontiguousarray(np.concatenate([W_hh[g] for g in gsl])),
            "b_hh": np.ascontiguousarray(np.concatenate([b_hh[g] for g in gsl])),
            "W_out": np.ascontiguousarray(woc),
            "b_out": np.ascontiguousarray(boc),
        })
    return in_maps


def _assemble(results):
    logp = np.concatenate([results[c]["out_logp"] for c in range(NC)])[:V]
    hnew = np.concatenate([results[c]["out_hnew"] for c in range(NC)])
    attnw = results[0]["out_attnw"]
    return (logp.reshape(1, V).astype(np.float32),
            hnew.reshape(1, 1, H).astype(np.float32),
            attnw.reshape(1, ML).astype(np.float32))


def kernel(**inputs):
    from concourse import bass_utils
    nc = _get_nc()
    in_maps = _per_core_inputs(inputs)
    res = bass_utils.run_bass_kernel_spmd(
        nc, in_maps, core_ids=list(range(NC)), trace=False)
    return _assemble(res.results)


if __name__ == "__main__":
    rng = np.random.default_rng(0)
    fake = {
        "input_tok": np.array([123], np.int64),
        "hidden": rng.standard_normal((1, 1, H), dtype=np.float32),
        "encoder_output": rng.standard_normal((1, H), dtype=np.float32),
        "encoder_outputs": rng.standard_normal((ML, H), dtype=np.float32),
        "emb": (rng.standard_normal((V, H), dtype=np.float32) * 0.02),
        "W_attn": (rng.standard_normal((ML, 2 * H), dtype=np.float32) * 0.02),
        "b_attn": np.zeros((ML,), np.float32),
        "W_comb": (rng.standard_normal((H, 2 * H), dtype=np.float32) * 0.02),
        "b_comb": np.zeros((H,), np.float32),
        "W_ih": (rng.standard_normal((3 * H, H), dtype=np.float32) * 0.02),
        "b_ih": np.zeros((3 * H,), np.float32),
        "W_hh": (rng.standard_normal((3 * H, H), dtype=np.float32) * 0.02),
        "b_hh": np.zeros((3 * H,), np.float32),
        "W_out": (rng.standard_normal((V, H), dtype=np.float32) * 0.02),
        "b_out": np.zeros((V,), np.float32),
    }
    outs = kernel(**fake)
    for o in outs:
        print(o.shape, o.dtype, float(np.abs(o).max()))
